# revision 15
# baseline (speedup 1.0000x reference)
"""Trainium2 Bass kernel for CustomQuantizedLinear.

Computes out[b,s,o] = sum_i x[b,s,i] * ((q[o,i]-128)*0.02) + bias[o]
for x (4,2048,4096) f32, q (4096,4096) int32, bias (4096,) f32.

Sharding across 8 NeuronCores: 4 token groups x 2 out-feature groups.
Each core computes a (2048 tokens, 2048 out-features) block of the
flattened (8192, 4096) output.

Host-side prep (layout only): x is cast to bf16 and w repacked to uint8
(lossless, values are 0..255), both pre-swizzled to [partition, row, ki]
layout so the contraction dim lands on SBUF partitions with no on-device
transposes.

Per-core dataflow:
  - w: DMA uint8 [128, o, ki] -> one ScalarE activation per 128-row
    slice dequantizes to resident bf16 tiles (Copy(q*0.02 - 2.56)).
  - x: DMA bf16 [128, tok, ki] slices per 128-token tile.
  - matmul: lhsT = xt[:, :, ki] (stationary, 128 tokens), rhs =
    wt[oc][:, :, ki] (moving, 512 out features), ki-outer / oc-inner so
    one stationary load feeds 4 N=512 matmuls into 4 PSUM banks.
  - weight prep is interleaved with token 0's per-oc matmul groups.
  - eviction: VectorE adds broadcast bias while copying PSUM->SBUF,
    then DMA out.
"""

import numpy as np

SCALE = 0.02
ZERO_POINT = 128

B, S, K, O = 4, 2048, 4096, 4096
N_CORES = 8
TOK_GROUPS, OUT_GROUPS = 4, 2
TOK_PC = B * S // TOK_GROUPS  # 2048 tokens per core
OUT_PC = O // OUT_GROUPS      # 2048 out features per core

_BUILD_CACHE = {}


def _build_bass(tok_pc=TOK_PC, out_pc=OUT_PC, k=K):
    """Build + compile the per-core Bass program. Returns (nc, names)."""
    from contextlib import ExitStack

    import concourse.mybir as mybir
    import concourse.tile as tile
    from concourse import bacc

    f32 = mybir.dt.float32
    bf16 = mybir.dt.bfloat16
    u8 = mybir.dt.uint8
    ADD = mybir.AluOpType.add
    Copy = mybir.ActivationFunctionType.Copy

    P = 128
    FREE = 512                 # matmul moving free dim (one PSUM bank of f32)
    KT = k // P                # number of k tiles
    TOKT = tok_pc // P         # number of token tiles
    OC = out_pc // FREE        # out chunks of 512
    OT_PER_OC = FREE // P      # w row tiles per out chunk

    nc = bacc.Bacc(None, target_bir_lowering=False)
    with tile.TileContext(nc) as tc:
        with ExitStack() as ctx:
            dram = ctx.enter_context(tc.tile_pool(name="dram", bufs=1, space="DRAM"))
            # pre-swizzled layouts: x [p, tok, ki] (contiguous DMA, strided
            # stationary is cheap); w [p, oc, ki, o'] (moving operand must
            # be contiguous)
            x_d = dram.tile([P, tok_pc, KT], bf16, kind="ExternalInput", name="x_in")
            w_d = dram.tile([P, OC, KT, FREE], u8, kind="ExternalInput", name="w_in")
            b_d = dram.tile([1, out_pc], f32, kind="ExternalInput", name="b_in")
            o_d = dram.tile([tok_pc, out_pc], f32, kind="ExternalOutput", name="o_out")

            const = ctx.enter_context(tc.tile_pool(name="const", bufs=1))
            stage = ctx.enter_context(tc.tile_pool(name="stage", bufs=3))
            wtp = ctx.enter_context(tc.tile_pool(name="wtp", bufs=1))
            xtp = ctx.enter_context(tc.tile_pool(name="xtp", bufs=3))
            outp = ctx.enter_context(tc.tile_pool(name="outp", bufs=4))
            psm = ctx.enter_context(tc.tile_pool(name="psm", bufs=8, space="PSUM"))

            # resident dequantized weights, split in K halves for finer
            # dependency gating: wt[oc*2+h] = [128, KT/2, 512 o] bf16
            KHALF = max(1, KT // 2)
            NW = (KT + KHALF - 1) // KHALF  # wt tiles per oc
            wt = [wtp.tile([P, KHALF, FREE], bf16, name=f"wt{j}")
                  for j in range(OC * NW)]

            def wt_rhs(oc, ki):
                return wt[oc * NW + ki // KHALF][:, ki % KHALF, :]

            KH = max(1, KT // 4)  # ki rows per prep slab
            deq_flip = [0]

            def prep_w(oc, kh, kh_size=None):
                """DMA + dequantize one [128, kh_size, 512] slab of w into wt."""
                sz = KH if kh_size is None else kh_size
                ki0 = kh * sz
                wstage = stage.tile([P, KH, FREE], u8, tag="stage",
                                    name=f"wst_{oc}_{kh}_{sz}")[:, :sz, :]
                nc.sync.dma_start(wstage, w_d[:, oc, ki0:ki0 + sz, :])
                dst = wt[oc * NW + ki0 // KHALF][
                    :, ki0 % KHALF:ki0 % KHALF + sz, :]
                # alternate dequant between ScalarE and VectorE
                if deq_flip[0] % 2 == 0:
                    nc.scalar.activation(
                        dst, wstage, Copy,
                        bias=float(-ZERO_POINT * SCALE), scale=float(SCALE))
                else:
                    nc.vector.tensor_scalar(
                        dst, wstage, float(SCALE), float(-ZERO_POINT * SCALE),
                        mybir.AluOpType.mult, mybir.AluOpType.add)
                deq_flip[0] += 1

            def make_xt(tt):
                xt = xtp.tile([P, P, KT], bf16, tag="xt", name=f"xt{tt}")
                nc.sync.dma_start(xt, x_d[:, tt * P:(tt + 1) * P, :])
                return xt

            def evict(tt, oc, acc):
                ot_sb = outp.tile([P, FREE], f32, tag="outt", name=f"o_{tt}_{oc}")
                nc.vector.tensor_tensor(
                    ot_sb, acc, bias_rep[:, oc * FREE:(oc + 1) * FREE], ADD)
                nc.sync.dma_start(
                    o_d[tt * P:(tt + 1) * P, oc * FREE:(oc + 1) * FREE], ot_sb)

            # token 0 + interleaved weight prep (prep runs one oc ahead of
            # the consuming matmul group)
            xt0 = make_xt(0)
            NSLAB = KT // KH
            KH0 = max(1, KH // 2)
            for kh in range(KT // KH0):
                prep_w(0, kh, KH0)
            # bias: replicate across partitions with a single broadcast DMA
            # (not needed until the first eviction, so emitted after the
            # critical first weight slabs)
            bias_rep = const.tile([P, out_pc], f32, name="bias_rep")
            nc.sync.dma_start(bias_rep, b_d[0, :].partition_broadcast(P))
            for oc in range(OC):
                if oc + 1 < OC:
                    for kh in range(NSLAB):
                        prep_w(oc + 1, kh)
                acc = psm.tile([P, FREE], f32, tag="acc", name=f"acc_0_{oc}")
                for ki in range(KT):
                    nc.tensor.matmul(
                        acc, lhsT=xt0[:, :, ki], rhs=wt_rhs(oc, ki),
                        start=(ki == 0), stop=(ki == KT - 1))
                evict(0, oc, acc)

            # remaining tokens: ki-outer / oc-inner (stationary reuse)
            for tt in range(1, TOKT):
                xt = make_xt(tt)
                accs = [psm.tile([P, FREE], f32, tag="acc", name=f"acc_{tt}_{oc}")
                        for oc in range(OC)]
                for ki in range(KT):
                    for oc in range(OC):
                        nc.tensor.matmul(
                            accs[oc], lhsT=xt[:, :, ki], rhs=wt_rhs(oc, ki),
                            start=(ki == 0), stop=(ki == KT - 1))
                for oc in range(OC):
                    evict(tt, oc, accs[oc])

            names = {
                "x": x_d.tensor.name,
                "w": w_d.tensor.name,
                "b": b_d.tensor.name,
                "o": o_d.tensor.name,
            }

    nc.compile()
    return nc, names


def _get_built(key=(TOK_PC, OUT_PC, K)):
    if key not in _BUILD_CACHE:
        _BUILD_CACHE[key] = _build_bass(*key)
    return _BUILD_CACHE[key]


def _swizzle(a2d, kt):
    """[rows, k] -> [128, rows, kt] with k = kt*128 split as (kt, 128)."""
    rows = a2d.shape[0]
    return np.ascontiguousarray(a2d.reshape(rows, kt, 128).transpose(2, 0, 1))


def _swizzle_w(q2d, kt, free=512):
    """[out, k] -> [128, out/free, kt, free] (w moving-operand layout)."""
    oc = q2d.shape[0] // free
    return np.ascontiguousarray(
        q2d.reshape(oc, free, kt, 128).transpose(3, 0, 2, 1))


def make_in_maps(x, quantized_weight, bias, names,
                 tok_pc=TOK_PC, out_pc=OUT_PC, k=K, n_cores=N_CORES,
                 out_groups=OUT_GROUPS):
    import ml_dtypes

    kt = k // 128
    bf16 = ml_dtypes.bfloat16
    xf = np.asarray(x, dtype=np.float32).reshape(-1, k).astype(bf16)
    w8 = np.asarray(quantized_weight).astype(np.uint8)
    bs = np.asarray(bias, dtype=np.float32)
    in_maps = []
    wsw = {}
    for c in range(n_cores):
        tg, og = divmod(c, out_groups)
        if og not in wsw:
            wsw[og] = _swizzle_w(w8[og * out_pc:(og + 1) * out_pc], kt)
        in_maps.append({
            names["x"]: _swizzle(xf[tg * tok_pc:(tg + 1) * tok_pc], kt),
            names["w"]: wsw[og],
            names["b"]: np.ascontiguousarray(
                bs[og * out_pc:(og + 1) * out_pc].reshape(1, out_pc)),
        })
    return in_maps


def assemble_out(results, names):
    out = np.empty((B * S, O), np.float32)
    for c, r in enumerate(results):
        tg, og = divmod(c, OUT_GROUPS)
        out[tg * TOK_PC:(tg + 1) * TOK_PC, og * OUT_PC:(og + 1) * OUT_PC] = \
            r[names["o"]]
    return out.reshape(B, S, O)


def kernel(x, quantized_weight, bias):
    from concourse.bass_utils import run_bass_kernel_spmd

    nc, names = _get_built()
    in_maps = make_in_maps(x, quantized_weight, bias, names)
    res = run_bass_kernel_spmd(nc, in_maps, core_ids=list(range(N_CORES)))
    return assemble_out(res.results, names)


# revision 18
# speedup vs baseline: 1.0047x; 1.0047x over previous
"""Trainium2 Bass kernel for CustomQuantizedLinear.

Computes out[b,s,o] = sum_i x[b,s,i] * ((q[o,i]-128)*0.02) + bias[o]
for x (4,2048,4096) f32, q (4096,4096) int32, bias (4096,) f32.

Sharding across 8 NeuronCores: 4 token groups x 2 out-feature groups.
Each core computes a (2048 tokens, 2048 out-features) block of the
flattened (8192, 4096) output.

Host-side prep (layout/dtype only): x is cast to bf16 and w repacked to
uint8 (lossless, values are 0..255), both pre-swizzled so the
contraction dim lands on SBUF partitions with no on-device transposes:
x -> [128, tok, ki] (contiguous DMA; the strided stationary read this
implies is cheap), w -> [128, oc, ki, o'] (the matmul moving operand
must be contiguous - a strided moving operand runs ~5x slower).

Per-core dataflow:
  - w: DMA uint8 slabs -> dequant to resident bf16 tiles, alternating
    ScalarE activation / VectorE tensor_scalar (Copy(q*0.02 - 2.56)).
  - x: one DMA per 128-token tile.
  - matmul: lhsT = xt[:, :, ki] (stationary, 128 tokens), rhs =
    wt(oc, ki) (moving, 512 out features), ki-outer / oc-inner so one
    stationary load feeds 4 N=512 matmuls into 4 PSUM banks; steady
    state runs at ~216 ns per matmul (PE warm at 2.4 GHz, LDWEIGHTS
    hidden).
  - weight prep is interleaved one oc ahead of token 0's matmul groups
    so the PE never sits in a separate prep phase.
  - eviction: VectorE adds the DMA-broadcast bias while copying
    PSUM->SBUF, then DMA out.

Measured on 8 axon trn2 cores: ~480-490 us HW exec vs a 437 us bf16
matmul roofline (8192x4096x4096 MACs / 8 cores @ 78.6 TFLOP/s).
"""

import numpy as np

SCALE = 0.02
ZERO_POINT = 128

B, S, K, O = 4, 2048, 4096, 4096
N_CORES = 8
TOK_GROUPS, OUT_GROUPS = 4, 2
TOK_PC = B * S // TOK_GROUPS  # 2048 tokens per core
OUT_PC = O // OUT_GROUPS      # 2048 out features per core

_BUILD_CACHE = {}


def _build_bass(tok_pc=TOK_PC, out_pc=OUT_PC, k=K):
    """Build + compile the per-core Bass program. Returns (nc, names)."""
    from contextlib import ExitStack

    import concourse.mybir as mybir
    import concourse.tile as tile
    from concourse import bacc

    f32 = mybir.dt.float32
    bf16 = mybir.dt.bfloat16
    u8 = mybir.dt.uint8
    ADD = mybir.AluOpType.add
    Copy = mybir.ActivationFunctionType.Copy

    P = 128
    FREE = 512                 # matmul moving free dim (one PSUM bank of f32)
    KT = k // P                # number of k tiles
    TOKT = tok_pc // P         # number of token tiles
    OC = out_pc // FREE        # out chunks of 512
    OT_PER_OC = FREE // P      # w row tiles per out chunk

    nc = bacc.Bacc(None, target_bir_lowering=False)
    with tile.TileContext(nc) as tc:
        with ExitStack() as ctx:
            dram = ctx.enter_context(tc.tile_pool(name="dram", bufs=1, space="DRAM"))
            # pre-swizzled layouts: x [p, tok, ki] (contiguous DMA, strided
            # stationary is cheap); w [p, oc, ki, o'] (moving operand must
            # be contiguous)
            x_d = dram.tile([P, tok_pc, KT], bf16, kind="ExternalInput", name="x_in")
            w_d = dram.tile([P, OC, KT, FREE], u8, kind="ExternalInput", name="w_in")
            b_d = dram.tile([1, out_pc], f32, kind="ExternalInput", name="b_in")
            o_d = dram.tile([tok_pc, out_pc], f32, kind="ExternalOutput", name="o_out")

            const = ctx.enter_context(tc.tile_pool(name="const", bufs=1))
            stage = ctx.enter_context(tc.tile_pool(name="stage", bufs=3))
            wtp = ctx.enter_context(tc.tile_pool(name="wtp", bufs=1))
            xtp = ctx.enter_context(tc.tile_pool(name="xtp", bufs=3))
            outp = ctx.enter_context(tc.tile_pool(name="outp", bufs=4))
            psm = ctx.enter_context(tc.tile_pool(name="psm", bufs=8, space="PSUM"))

            # resident dequantized weights, split in K halves for finer
            # dependency gating: wt[oc*2+h] = [128, KT/2, 512 o] bf16
            KHALF = max(1, KT // 2)
            NW = (KT + KHALF - 1) // KHALF  # wt tiles per oc
            wt = [wtp.tile([P, KHALF, FREE], bf16, name=f"wt{j}")
                  for j in range(OC * NW)]

            def wt_rhs(oc, ki):
                return wt[oc * NW + ki // KHALF][:, ki % KHALF, :]

            KH = max(1, KT // 4)  # ki rows per prep slab
            deq_flip = [0]

            def prep_w(oc, kh, kh_size=None):
                """DMA + dequantize one [128, kh_size, 512] slab of w into wt."""
                sz = KH if kh_size is None else kh_size
                ki0 = kh * sz
                wstage = stage.tile([P, KH, FREE], u8, tag="stage",
                                    name=f"wst_{oc}_{kh}_{sz}")[:, :sz, :]
                nc.sync.dma_start(wstage, w_d[:, oc, ki0:ki0 + sz, :])
                dst = wt[oc * NW + ki0 // KHALF][
                    :, ki0 % KHALF:ki0 % KHALF + sz, :]
                # alternate dequant between ScalarE and VectorE
                if deq_flip[0] % 2 == 0:
                    nc.scalar.activation(
                        dst, wstage, Copy,
                        bias=float(-ZERO_POINT * SCALE), scale=float(SCALE))
                else:
                    nc.vector.tensor_scalar(
                        dst, wstage, float(SCALE), float(-ZERO_POINT * SCALE),
                        mybir.AluOpType.mult, mybir.AluOpType.add)
                deq_flip[0] += 1

            def make_xt(tt):
                xt = xtp.tile([P, P, KT], bf16, tag="xt", name=f"xt{tt}")
                nc.sync.dma_start(xt, x_d[:, tt * P:(tt + 1) * P, :])
                return xt

            def evict(tt, oc, acc):
                ot_sb = outp.tile([P, FREE], f32, tag="outt", name=f"o_{tt}_{oc}")
                nc.vector.tensor_tensor(
                    ot_sb, acc, bias_rep[:, oc * FREE:(oc + 1) * FREE], ADD)
                nc.sync.dma_start(
                    o_d[tt * P:(tt + 1) * P, oc * FREE:(oc + 1) * FREE], ot_sb)

            # token 0 + interleaved weight prep (prep runs one oc ahead of
            # the consuming matmul group)
            xt0 = make_xt(0)
            NSLAB = KT // KH
            KH0 = max(1, KH // 2)
            for kh in range(KT // KH0):
                prep_w(0, kh, KH0)
            # bias: replicate across partitions with a single broadcast DMA
            # (not needed until the first eviction, so emitted after the
            # critical first weight slabs)
            bias_rep = const.tile([P, out_pc], f32, name="bias_rep")
            nc.sync.dma_start(bias_rep, b_d[0, :].partition_broadcast(P))
            for oc in range(OC):
                if oc + 1 < OC:
                    for kh in range(NSLAB):
                        prep_w(oc + 1, kh)
                acc = psm.tile([P, FREE], f32, tag="acc", name=f"acc_0_{oc}")
                for ki in range(KT):
                    nc.tensor.matmul(
                        acc, lhsT=xt0[:, :, ki], rhs=wt_rhs(oc, ki),
                        start=(ki == 0), stop=(ki == KT - 1))
                evict(0, oc, acc)

            # remaining tokens: ki-outer / oc-inner (stationary reuse)
            for tt in range(1, TOKT):
                xt = make_xt(tt)
                accs = [psm.tile([P, FREE], f32, tag="acc", name=f"acc_{tt}_{oc}")
                        for oc in range(OC)]
                for ki in range(KT):
                    for oc in range(OC):
                        nc.tensor.matmul(
                            accs[oc], lhsT=xt[:, :, ki], rhs=wt_rhs(oc, ki),
                            start=(ki == 0), stop=(ki == KT - 1))
                for oc in range(OC):
                    evict(tt, oc, accs[oc])

            names = {
                "x": x_d.tensor.name,
                "w": w_d.tensor.name,
                "b": b_d.tensor.name,
                "o": o_d.tensor.name,
            }

    nc.compile()
    return nc, names


def _get_built(key=(TOK_PC, OUT_PC, K)):
    if key not in _BUILD_CACHE:
        _BUILD_CACHE[key] = _build_bass(*key)
    return _BUILD_CACHE[key]


def _swizzle(a2d, kt):
    """[rows, k] -> [128, rows, kt] with k = kt*128 split as (kt, 128)."""
    rows = a2d.shape[0]
    return np.ascontiguousarray(a2d.reshape(rows, kt, 128).transpose(2, 0, 1))


def _swizzle_w(q2d, kt, free=512):
    """[out, k] -> [128, out/free, kt, free] (w moving-operand layout)."""
    oc = q2d.shape[0] // free
    return np.ascontiguousarray(
        q2d.reshape(oc, free, kt, 128).transpose(3, 0, 2, 1))


def make_in_maps(x, quantized_weight, bias, names,
                 tok_pc=TOK_PC, out_pc=OUT_PC, k=K, n_cores=N_CORES,
                 out_groups=OUT_GROUPS):
    import ml_dtypes

    kt = k // 128
    bf16 = ml_dtypes.bfloat16
    xf = np.asarray(x, dtype=np.float32).reshape(-1, k).astype(bf16)
    w8 = np.asarray(quantized_weight).astype(np.uint8)
    bs = np.asarray(bias, dtype=np.float32)
    in_maps = []
    wsw = {}
    for c in range(n_cores):
        tg, og = divmod(c, out_groups)
        if og not in wsw:
            wsw[og] = _swizzle_w(w8[og * out_pc:(og + 1) * out_pc], kt)
        in_maps.append({
            names["x"]: _swizzle(xf[tg * tok_pc:(tg + 1) * tok_pc], kt),
            names["w"]: wsw[og],
            names["b"]: np.ascontiguousarray(
                bs[og * out_pc:(og + 1) * out_pc].reshape(1, out_pc)),
        })
    return in_maps


def assemble_out(results, names):
    out = np.empty((B * S, O), np.float32)
    for c, r in enumerate(results):
        tg, og = divmod(c, OUT_GROUPS)
        out[tg * TOK_PC:(tg + 1) * TOK_PC, og * OUT_PC:(og + 1) * OUT_PC] = \
            r[names["o"]]
    return out.reshape(B, S, O)


def kernel(x, quantized_weight, bias):
    from concourse.bass_utils import run_bass_kernel_spmd

    nc, names = _get_built()
    in_maps = make_in_maps(x, quantized_weight, bias, names)
    res = run_bass_kernel_spmd(nc, in_maps, core_ids=list(range(N_CORES)))
    return assemble_out(res.results, names)


# revision 21
# speedup vs baseline: 1.0438x; 1.0390x over previous
"""Trainium2 Bass kernel for CustomQuantizedLinear.

Computes out[b,s,o] = sum_i x[b,s,i] * ((q[o,i]-128)*0.02) + bias[o]
for x (4,2048,4096) f32, q (4096,4096) int32, bias (4096,) f32.

Sharding across 8 NeuronCores: column-parallel (8 out-feature groups,
x replicated). Each core computes a (8192 tokens, 512 out-features)
block of the flattened (8192, 4096) output; weight prep per core is
tiny (4.2 MB uint8) so matmuls start ~15 us in and the PE clock stays
warm for the whole run.

Host-side prep (layout/dtype only): x is cast to bf16 and w repacked to
uint8 (lossless, values are 0..255), both pre-swizzled so the
contraction dim lands on SBUF partitions with no on-device transposes:
x -> [128, tok, ki] (contiguous DMA; the strided stationary read this
implies is cheap), w -> [128, oc, ki, o'] (the matmul moving operand
must be contiguous - a strided moving operand runs ~5x slower).

Per-core dataflow:
  - w: DMA uint8 slabs -> dequant to resident bf16 tiles, alternating
    ScalarE activation / VectorE tensor_scalar (Copy(q*0.02 - 2.56)).
  - x: one DMA per 128-token tile.
  - matmul: lhsT = xt[:, :, ki] (stationary, 128 tokens), rhs =
    wt(oc, ki) (moving, 512 out features), ki-outer / oc-inner so one
    stationary load feeds 4 N=512 matmuls into 4 PSUM banks; steady
    state runs at ~216 ns per matmul (PE warm at 2.4 GHz, LDWEIGHTS
    hidden).
  - weight prep is interleaved one oc ahead of token 0's matmul groups
    so the PE never sits in a separate prep phase.
  - eviction: VectorE adds the DMA-broadcast bias while copying
    PSUM->SBUF, then DMA out.

Measured on 8 axon trn2 cores: ~468 us HW exec vs a 437 us bf16
matmul roofline (8192x4096x4096 MACs / 8 cores @ 78.6 TFLOP/s).
"""

import numpy as np

SCALE = 0.02
ZERO_POINT = 128

B, S, K, O = 4, 2048, 4096, 4096
N_CORES = 8
TOK_GROUPS, OUT_GROUPS = 1, 8
TOK_PC = B * S // TOK_GROUPS  # 2048 tokens per core
OUT_PC = O // OUT_GROUPS      # 2048 out features per core

_BUILD_CACHE = {}


def _build_bass(tok_pc=TOK_PC, out_pc=OUT_PC, k=K):
    """Build + compile the per-core Bass program. Returns (nc, names)."""
    from contextlib import ExitStack

    import concourse.mybir as mybir
    import concourse.tile as tile
    from concourse import bacc

    f32 = mybir.dt.float32
    bf16 = mybir.dt.bfloat16
    u8 = mybir.dt.uint8
    ADD = mybir.AluOpType.add
    Copy = mybir.ActivationFunctionType.Copy

    P = 128
    FREE = 512                 # matmul moving free dim (one PSUM bank of f32)
    KT = k // P                # number of k tiles
    TOKT = tok_pc // P         # number of token tiles
    OC = out_pc // FREE        # out chunks of 512
    OT_PER_OC = FREE // P      # w row tiles per out chunk

    nc = bacc.Bacc(None, target_bir_lowering=False)
    with tile.TileContext(nc) as tc:
        with ExitStack() as ctx:
            dram = ctx.enter_context(tc.tile_pool(name="dram", bufs=1, space="DRAM"))
            # pre-swizzled layouts: x [p, tok, ki] (contiguous DMA, strided
            # stationary is cheap); w [p, oc, ki, o'] (moving operand must
            # be contiguous)
            x_d = dram.tile([P, tok_pc, KT], bf16, kind="ExternalInput", name="x_in")
            w_d = dram.tile([P, OC, KT, FREE], u8, kind="ExternalInput", name="w_in")
            b_d = dram.tile([1, out_pc], f32, kind="ExternalInput", name="b_in")
            o_d = dram.tile([tok_pc, out_pc], f32, kind="ExternalOutput", name="o_out")

            const = ctx.enter_context(tc.tile_pool(name="const", bufs=1))
            stage = ctx.enter_context(tc.tile_pool(name="stage", bufs=3))
            wtp = ctx.enter_context(tc.tile_pool(name="wtp", bufs=1))
            xtp = ctx.enter_context(tc.tile_pool(name="xtp", bufs=3))
            outp = ctx.enter_context(tc.tile_pool(name="outp", bufs=4))
            psm = ctx.enter_context(tc.tile_pool(name="psm", bufs=8, space="PSUM"))

            # resident dequantized weights, split in K quarters for finer
            # dependency gating
            KHALF = max(1, KT // 4)
            NW = (KT + KHALF - 1) // KHALF  # wt tiles per oc
            wt = [wtp.tile([P, KHALF, FREE], bf16, name=f"wt{j}")
                  for j in range(OC * NW)]

            def wt_rhs(oc, ki):
                return wt[oc * NW + ki // KHALF][:, ki % KHALF, :]

            KH = max(1, KT // 4)  # ki rows per prep slab
            deq_flip = [0]

            def prep_w(oc, kh, kh_size=None):
                """DMA + dequantize one [128, kh_size, 512] slab of w into wt."""
                sz = KH if kh_size is None else kh_size
                ki0 = kh * sz
                wstage = stage.tile([P, KH, FREE], u8, tag="stage",
                                    name=f"wst_{oc}_{kh}_{sz}")[:, :sz, :]
                nc.sync.dma_start(wstage, w_d[:, oc, ki0:ki0 + sz, :])
                dst = wt[oc * NW + ki0 // KHALF][
                    :, ki0 % KHALF:ki0 % KHALF + sz, :]
                # alternate dequant between ScalarE and VectorE
                if deq_flip[0] % 2 == 0:
                    nc.scalar.activation(
                        dst, wstage, Copy,
                        bias=float(-ZERO_POINT * SCALE), scale=float(SCALE))
                else:
                    nc.vector.tensor_scalar(
                        dst, wstage, float(SCALE), float(-ZERO_POINT * SCALE),
                        mybir.AluOpType.mult, mybir.AluOpType.add)
                deq_flip[0] += 1

            def make_xt(tt):
                xt = xtp.tile([P, P, KT], bf16, tag="xt", name=f"xt{tt}")
                nc.sync.dma_start(xt, x_d[:, tt * P:(tt + 1) * P, :])
                return xt

            def evict(tt, oc, acc):
                ot_sb = outp.tile([P, FREE], f32, tag="outt", name=f"o_{tt}_{oc}")
                nc.vector.tensor_tensor(
                    ot_sb, acc, bias_rep[:, oc * FREE:(oc + 1) * FREE], ADD)
                nc.sync.dma_start(
                    o_d[tt * P:(tt + 1) * P, oc * FREE:(oc + 1) * FREE], ot_sb)

            # token 0 + interleaved weight prep (prep runs one oc ahead of
            # the consuming matmul group)
            xt0 = make_xt(0)
            NSLAB = KT // KH
            KH0 = max(1, KH // 2)
            for kh in range(KT // KH0):
                prep_w(0, kh, KH0)
            # bias: replicate across partitions with a single broadcast DMA
            # (not needed until the first eviction, so emitted after the
            # critical first weight slabs)
            bias_rep = const.tile([P, out_pc], f32, name="bias_rep")
            nc.sync.dma_start(bias_rep, b_d[0, :].partition_broadcast(P))
            for oc in range(OC):
                if oc + 1 < OC:
                    for kh in range(NSLAB):
                        prep_w(oc + 1, kh)
                acc = psm.tile([P, FREE], f32, tag="acc", name=f"acc_0_{oc}")
                for ki in range(KT):
                    nc.tensor.matmul(
                        acc, lhsT=xt0[:, :, ki], rhs=wt_rhs(oc, ki),
                        start=(ki == 0), stop=(ki == KT - 1))
                evict(0, oc, acc)

            # remaining tokens: ki-outer / oc-inner (stationary reuse)
            for tt in range(1, TOKT):
                xt = make_xt(tt)
                accs = [psm.tile([P, FREE], f32, tag="acc", name=f"acc_{tt}_{oc}")
                        for oc in range(OC)]
                for ki in range(KT):
                    for oc in range(OC):
                        nc.tensor.matmul(
                            accs[oc], lhsT=xt[:, :, ki], rhs=wt_rhs(oc, ki),
                            start=(ki == 0), stop=(ki == KT - 1))
                for oc in range(OC):
                    evict(tt, oc, accs[oc])

            names = {
                "x": x_d.tensor.name,
                "w": w_d.tensor.name,
                "b": b_d.tensor.name,
                "o": o_d.tensor.name,
            }

    nc.compile()
    return nc, names


def _get_built(key=(TOK_PC, OUT_PC, K)):
    if key not in _BUILD_CACHE:
        _BUILD_CACHE[key] = _build_bass(*key)
    return _BUILD_CACHE[key]


def _swizzle(a2d, kt):
    """[rows, k] -> [128, rows, kt] with k = kt*128 split as (kt, 128)."""
    rows = a2d.shape[0]
    return np.ascontiguousarray(a2d.reshape(rows, kt, 128).transpose(2, 0, 1))


def _swizzle_w(q2d, kt, free=512):
    """[out, k] -> [128, out/free, kt, free] (w moving-operand layout)."""
    oc = q2d.shape[0] // free
    return np.ascontiguousarray(
        q2d.reshape(oc, free, kt, 128).transpose(3, 0, 2, 1))


def make_in_maps(x, quantized_weight, bias, names,
                 tok_pc=TOK_PC, out_pc=OUT_PC, k=K, n_cores=N_CORES,
                 out_groups=OUT_GROUPS):
    import ml_dtypes

    kt = k // 128
    bf16 = ml_dtypes.bfloat16
    xf = np.asarray(x, dtype=np.float32).reshape(-1, k).astype(bf16)
    w8 = np.asarray(quantized_weight).astype(np.uint8)
    bs = np.asarray(bias, dtype=np.float32)
    in_maps = []
    wsw = {}
    xsw = {}
    for c in range(n_cores):
        tg, og = divmod(c, out_groups)
        if og not in wsw:
            wsw[og] = _swizzle_w(w8[og * out_pc:(og + 1) * out_pc], kt)
        if tg not in xsw:
            xsw[tg] = _swizzle(xf[tg * tok_pc:(tg + 1) * tok_pc], kt)
        in_maps.append({
            names["x"]: xsw[tg],
            names["w"]: wsw[og],
            names["b"]: np.ascontiguousarray(
                bs[og * out_pc:(og + 1) * out_pc].reshape(1, out_pc)),
        })
    return in_maps


def assemble_out(results, names):
    out = np.empty((B * S, O), np.float32)
    for c, r in enumerate(results):
        tg, og = divmod(c, OUT_GROUPS)
        out[tg * TOK_PC:(tg + 1) * TOK_PC, og * OUT_PC:(og + 1) * OUT_PC] = \
            r[names["o"]]
    return out.reshape(B, S, O)


def kernel(x, quantized_weight, bias):
    from concourse.bass_utils import run_bass_kernel_spmd

    nc, names = _get_built()
    in_maps = make_in_maps(x, quantized_weight, bias, names)
    res = run_bass_kernel_spmd(nc, in_maps, core_ids=list(range(N_CORES)))
    return assemble_out(res.results, names)
